# revision 1
# baseline (speedup 1.0000x reference)
"""Trainium2 Bass kernel for nn_CausalSelfAttention (tensor-parallel over heads, 8 cores).

Contract: kernel(**inputs) takes FULL unsharded numpy inputs and returns the
FULL output [1, 2048, 1024] float32. Internally: shards over 8 NeuronCores
(2 heads each, Wq/Wk/Wv column-sharded, Wo row-sharded), runs one SPMD Bass
program via run_bass_kernel_spmd, and sums the 8 partial Wo products on the
host (the row-parallel unshard).

v2 design (ACT-exp is the per-block bottleneck; everything else is arranged
to stay off the Scalar engine and off the critical chain):
  - weights pre-transposed on host to [p, (i f)] so every DRAM load is
    contiguous 2KB-per-partition (fast descriptors); projections ordered
    q -> k -> v so the softmax-path ACT work starts earliest
  - cosine-norm: sumsq matmul (hselw) -> Ln/Exp on ACT (chunk-0 fine-grained,
    chunks 1-3 batched); rotary as (raw*rota + swap(raw)*rotb)*rw with the
    partition swap done by sync-queue SBUF-to-SBUF DMA on raw (off the
    rw critical path; swap(rw)==rw since rw is constant within a head)
  - v to natural layout via XBAR DMA transpose (no PE/DVE involvement);
    vext per head h0=[v|ones], h1=[ones|v] so y and Z land on partition
    halves that keep every later DVE op partition-aligned
  - attention: S^T row-packed pairs (concurrent in PE), EXP on ACT (bf16),
    causal diagonal mask via gpsimd affine_select, AV col-packed with the
    ones rows producing the softmax denominator Z in the same matmul
  - tails: 1/Z via reciprocal_approx_fast (custom DVE, ~5x), cross-partition
    Z alignment via SBUF-to-SBUF DMA, fused y*(1/Z) from PSUM, Wo matmuls,
    DVE copy PSUM->SBUF, DMA out fp32
"""

import os
import sys
import types

import numpy as np
import ml_dtypes

for _p in ("/opt/trn_rl_repo", "/root/.axon_site/_ro/trn_rl_repo"):
    if os.path.isdir(_p) and _p not in sys.path:
        sys.path.append(_p)

import concourse.bass as bass
import concourse.mybir as mybir
import concourse.tile as tile
from concourse.bass_utils import run_bass_kernel_spmd

F32 = mybir.dt.float32
BF16 = mybir.dt.bfloat16
NPBF16 = ml_dtypes.bfloat16
NCORES = 8
T = 2048
D = 1024
NH = 16
HD = 64
HPC = NH // NCORES   # heads per core
EPC = HPC * HD       # projection cols per core
ATTN_SCALE = 0.12
NT = T // 512
NK = D // 128

LAST = {}


def _register_ntff_hook():
    """Best-effort: register the axon NTFF profile hook if the image's antenv
    lacks axon_hooks (profiling only; compile/run work without it)."""
    try:
        import antenv.axon_hooks  # noqa: F401
        return
    except ImportError:
        pass
    try:
        import trn_agent_boot.trn_boot as tb

        mod = types.ModuleType("antenv.axon_hooks")
        holder = {}
        mod.set_axon_ntff_profile_hook = lambda h: holder.__setitem__("h", h)
        mod.get_axon_ntff_profile_hook = lambda: holder.get("h")
        sys.modules["antenv.axon_hooks"] = mod
        mod.set_axon_ntff_profile_hook(
            tb._ntff_profile_via_ctypes("/opt/axon/libaxon_pjrt.so")
        )
    except Exception:
        pass


def _split_ctrl_waits(nc, k_default=1):
    """The container's walrus build rejects instructions carrying more than one
    semaphore sync-wait; hoist extra waits onto single-wait NoOps that precede
    the instruction on the same engine queue (AND semantics preserved)."""
    n_nops = 0
    for f in nc.m.functions:
        for blk in f.blocks:
            new, changed = [], False
            for inst in list(blk.instructions):
                si = inst.sync_info
                waits = list(si.on_wait) if si is not None else []
                kmax = 1 if isinstance(inst, mybir.InstDrain) else k_default
                if len(waits) > kmax:
                    for k, w in enumerate(waits[:-kmax]):
                        nop = mybir.InstNoOp(name=f"{inst.name}-sw{k}", ins=[], outs=[])
                        nop.engine = inst.engine
                        nop.sync_info = mybir.SyncInfo(on_wait=[w], on_update=[])
                        new.append(nop)
                        n_nops += 1
                    inst.sync_info = mybir.SyncInfo(
                        on_wait=list(waits[-kmax:]), on_update=list(si.on_update)
                    )
                    changed = True
                new.append(inst)
            if changed:
                blk.instructions = new
    return n_nops


def _build_nc():
    nc = bass.Bass("TRN2", target_bir_lowering=False, debug=False, num_devices=NCORES)

    xT_d = nc.dram_tensor("xT", [D, T], BF16, kind="ExternalInput")
    wq_d = nc.dram_tensor("wq", [128, D], BF16, kind="ExternalInput")
    wk_d = nc.dram_tensor("wk", [128, D], BF16, kind="ExternalInput")
    wv_d = nc.dram_tensor("wv", [128, D], BF16, kind="ExternalInput")
    wo_d = nc.dram_tensor("wo", [EPC, D], BF16, kind="ExternalInput")
    rota_d = nc.dram_tensor("rota", [EPC, T], BF16, kind="ExternalInput")
    rotb_d = nc.dram_tensor("rotb", [EPC, T], BF16, kind="ExternalInput")
    hselw_d = nc.dram_tensor("hselw", [128, 128], BF16, kind="ExternalInput")
    hswap_d = nc.dram_tensor("hswap", [128, 128], BF16, kind="ExternalInput")
    out_d = nc.dram_tensor("out", [T, D], F32, kind="ExternalOutput")

    with tile.TileContext(nc) as tc:
        with (
            tc.tile_pool(name="wt", bufs=1) as wt,
            tc.tile_pool(name="big", bufs=8) as big,      # xT chunks
            tc.tile_pool(name="praw", bufs=3) as prawp,   # q/k/v raw
            tc.tile_pool(name="srawp", bufs=2) as srawp,  # swapped raw q/k
            tc.tile_pool(name="lwp", bufs=2) as lwp,      # ln(sumsq) staging
            tc.tile_pool(name="rwp", bufs=2) as rwp,      # rsqrt scales
            tc.tile_pool(name="rotp", bufs=2) as rotp,    # qrot/krot
            tc.tile_pool(name="sm", bufs=2) as smp,       # misc small tiles
            tc.tile_pool(name="at", bufs=1) as atp,       # pt2 / yt / ost
            tc.tile_pool(name="ps", bufs=2, space="PSUM") as psp,    # [128,1024] x2
            tc.tile_pool(name="sqp", bufs=2, space="PSUM") as sqp,   # [128,512] x2
            tc.tile_pool(name="py", bufs=2, space="PSUM") as pyp,    # pyh 2 tags
        ):
            # ---- ACT table warmup: force the Ln/Exp table load at t~0 ----
            eps = wt.tile([128, 1], F32, tag="eps")
            nc.gpsimd.memset(eps[:], 1e-12)
            warm_in = wt.tile([128, 2], F32, tag="warm_in")
            warm_out = wt.tile([128, 2], F32, tag="warm_out")
            nc.gpsimd.memset(warm_in[:], 1.0)
            nc.scalar.activation(
                warm_out[:], warm_in[:], mybir.ActivationFunctionType.Ln,
                bias=eps[:],
            )
            nc.scalar.activation(
                warm_in[:], warm_out[:], mybir.ActivationFunctionType.Exp
            )

            # ---- constants / weights (all DRAM layouts are contiguous) ----
            wq_s = wt.tile([128, D], BF16, tag="wq")
            wk_s = wt.tile([128, D], BF16, tag="wk")
            wv_s = wt.tile([128, D], BF16, tag="wv")
            wo_s = wt.tile([EPC, D], BF16, tag="wo")
            rota = wt.tile([EPC, T], BF16, tag="rota")
            rotb = wt.tile([EPC, T], BF16, tag="rotb")
            hselw = wt.tile([128, 128], BF16, tag="hselw")
            hswap = wt.tile([128, 128], BF16, tag="hswap")
            vext = wt.tile([128, (T // 128) * 256], BF16, tag="vext")

            xc = []
            for i in range(NK):
                t_ = big.tile([128, T], BF16, tag="big")
                nc.sync.dma_start(t_[:], xT_d[128 * i : 128 * (i + 1), :])
                xc.append(t_)
                if i == 0:
                    nc.sync.dma_start(wq_s[:], wq_d[:])
                if i == 1:
                    nc.sync.dma_start(wk_s[:], wk_d[:])
                if i == 2:
                    nc.sync.dma_start(hselw[:], hselw_d[:])
                    nc.sync.dma_start(hswap[:], hswap_d[:])
                if i == 4:
                    nc.sync.dma_start(wv_s[:], wv_d[:])
            nc.sync.dma_start(wo_s[:], wo_d[:])
            nc.sync.dma_start(rota[:], rota_d[:])
            nc.sync.dma_start(rotb[:], rotb_d[:])
            nc.gpsimd.memset(vext[:], 1.0)

            # ---- projections: transposed layout [EPC, T], bf16 out ----
            def project(w_s, pname):
                # contraction-outer: the first matmul needs only x chunk 0,
                # so PE starts as soon as the first 512KB of x lands
                raw = prawp.tile([EPC, T], BF16, tag="praw", name=f"raw_{pname}")
                pspair = [
                    psp.tile([128, 1024], F32, tag="ps", name=f"pj_{pname}_{p}")
                    for p in range(2)
                ]
                for i in range(NK):
                    for n in range(NT):
                        nc.tensor.matmul(
                            pspair[n // 2][:, 512 * (n % 2) : 512 * (n % 2 + 1)],
                            w_s[:, 128 * i : 128 * (i + 1)],
                            xc[i][:, 512 * n : 512 * (n + 1)],
                            start=(i == 0),
                            stop=(i == NK - 1),
                        )
                for n in range(NT):
                    nc.vector.tensor_copy(
                        raw[:, 512 * n : 512 * (n + 1)],
                        pspair[n // 2][:, 512 * (n % 2) : 512 * (n % 2 + 1)],
                    )
                return raw

            # ---- cosine-norm + scale + rotary for one tensor ----
            # rsqrt(sumsq) = exp(-0.5 ln(sumsq)); both Ln and Exp live in the
            # same ACT table set as the attention Exp (no table switch).
            # rot = (raw*rota + swap(raw)*rotb) * rw  -- swap(rw)==rw because
            # hselw broadcasts the per-head sumsq to all 64 head rows.
            def norm_rot(raw, tname):
                # swap(raw) comes from a PE permutation matmul (hswap) into
                # PSUM; the t2 multiply reads it straight from there.
                lw = lwp.tile([128, T], F32, tag="lw", name=f"lw_{tname}")
                sws = []
                for n in range(NT):
                    sl = slice(512 * n, 512 * (n + 1))
                    sw = sqp.tile([128, 512], F32, name=f"sw_{tname}_{n}", tag="sq")
                    nc.tensor.matmul(sw[:], hswap[:], raw[:, sl], start=True,
                                     stop=True)
                    sws.append(sw)
                    sq = smp.tile([128, 512], BF16, name=f"sq_{tname}_{n}", tag="sq",
                                  bufs=4)
                    nc.gpsimd.tensor_mul(sq[:], raw[:, sl], raw[:, sl])
                    ssb = sqp.tile([128, 512], F32, name=f"ssb_{tname}_{n}", tag="sq")
                    nc.tensor.matmul(ssb[:], hselw[:], sq[:], start=True, stop=True)
                    nc.scalar.activation(
                        lw[:, sl], ssb[:], mybir.ActivationFunctionType.Ln,
                        bias=eps[:],
                    )
                    # t2 frees the swap PSUM slot quickly (sq tag has bufs=2)
                    t2 = smp.tile([128, 512], BF16, name=f"t2_{tname}_{n}", tag="t2",
                                  bufs=4)
                    nc.vector.tensor_mul(t2[:], sw[:], rotb[:, sl])
                    sws[n] = t2
                rw = rwp.tile([128, T], BF16, tag="rw", name=f"rw_{tname}")
                # chunk 0 alone (unblocks attention earliest), chunks 1-3 batched
                nc.scalar.activation(
                    rw[:, 0:512], lw[:, 0:512],
                    mybir.ActivationFunctionType.Exp, scale=-0.5,
                )
                nc.scalar.activation(
                    rw[:, 512:T], lw[:, 512:T],
                    mybir.ActivationFunctionType.Exp, scale=-0.5,
                )
                rot = rotp.tile([EPC, T], BF16, tag="rot", name=f"rot_{tname}")
                for n in range(NT):
                    sl = slice(512 * n, 512 * (n + 1))
                    t1 = smp.tile([128, 512], BF16, name=f"t1_{tname}_{n}", tag="t1",
                                  bufs=4)
                    nc.vector.tensor_mul(t1[:], raw[:, sl], rota[:, sl])
                    nc.vector.tensor_add(t1[:], t1[:], sws[n][:])
                    nc.vector.tensor_mul(rot[:, sl], t1[:], rw[:, sl])
                return rot

            q_raw = project(wq_s, "q")
            qrot = norm_rot(q_raw, "q")
            k_raw = project(wk_s, "k")
            krot = norm_rot(k_raw, "k")
            vT_raw = project(wv_s, "v")

            # ---- v to natural layout via PE transpose ----
            # vext per 128-chunk j: h0 slot [v|ones], h1 slot [ones|v] so the
            # AV matmul puts y0 on partitions 0-63 / Z0 on 64-127 but
            # Z1 on 0-63 / y1 on 64-127 -- every later elementwise op is then
            # partition-aligned with its operands.
            ident = wt.tile([128, 128], BF16, tag="ident")
            nc.gpsimd.memset(ident[:], 0.0)
            nc.gpsimd.affine_select(
                out=ident[:],
                in_=ident[:],
                compare_op=mybir.AluOpType.not_equal,
                fill=1.0,
                base=0,
                pattern=[[-1, 128]],
                channel_multiplier=1,
            )
            for j in range(T // 128):
                tp_ = sqp.tile([128, 128], BF16, tag="sq", name=f"vtp_{j}")
                nc.tensor.transpose(
                    tp_[:], vT_raw[:, 128 * j : 128 * (j + 1)], ident[:]
                )
                nc.vector.tensor_copy(
                    vext[:, 256 * j : 256 * j + 64], tp_[:, 0:64]
                )
                nc.vector.tensor_copy(
                    vext[:, 256 * j + 192 : 256 * j + 256], tp_[:, 64:128]
                )

            # ---- attention (S^T layout) + Wo partial ----
            # chunk tails (normalize + Wo + store) are emitted AFTER the next
            # chunk's attention core so the PE queue never stalls on tail work
            def attention_core(c):
                # Software-pipelined: the S matmul pair for block j+PF is
                # emitted BEFORE mask/AV of block j, so in the PE's FIFO the
                # S work for upcoming blocks sits ahead of AV(j) (which waits
                # on exp+mask). This keeps PE/ACT/GpSimd all streaming and,
                # at chunk boundaries, lets the next chunk's S/exp proceed
                # while the previous tail (reciprocal etc.) drains pyh.
                PF = 4
                nts = 4 * c + 4
                cq = slice(512 * c, 512 * (c + 1))
                pyh = [
                    pyp.tile([128, 512], F32, name=f"py0_{c}", tag="py0", bufs=1),
                    pyp.tile([128, 512], F32, name=f"py1_{c}", tag="py1", bufs=1),
                ]
                ps_tiles = [None] * nts
                pt_tiles = [None] * nts

                def emit_s(j):
                    ps2 = psp.tile([128, 1024], F32, tag="ps", name=f"s_{c}_{j}")
                    for h in range(HPC):
                        hs = slice(64 * h, 64 * (h + 1))
                        nc.tensor.matmul(
                            ps2[:, 512 * h : 512 * (h + 1)],
                            krot[hs, 128 * j : 128 * (j + 1)],
                            qrot[hs, cq],
                            start=True,
                            stop=True,
                            tile_position=(64 * h, 0),
                        )
                    ps_tiles[j] = ps2

                def emit_exp(j):
                    pt2 = atp.tile([128, 1024], BF16, tag="pt", bufs=5,
                                   name=f"p_{c}_{j}")
                    nc.scalar.activation(
                        pt2[:], ps_tiles[j][:], mybir.ActivationFunctionType.Exp,
                        scale=ATTN_SCALE,
                    )
                    pt_tiles[j] = pt2

                for j in range(min(PF, nts)):
                    emit_s(j)
                    emit_exp(j)
                for j in range(nts):
                    if j + PF < nts:
                        emit_s(j + PF)
                        emit_exp(j + PF)
                    m = j - 4 * c
                    pt2 = pt_tiles[j]
                    if m >= 0:
                        # causal: keep pt[x, (h, y)] only where y >= x + 128*m
                        nc.gpsimd.affine_select(
                            out=pt2[:],
                            in_=pt2[:],
                            compare_op=mybir.AluOpType.is_ge,
                            fill=0.0,
                            base=-128 * m,
                            pattern=[[0, 2], [1, 512]],
                            channel_multiplier=-1,
                        )
                    for h in range(HPC):
                        nc.tensor.matmul(
                            pyh[h][:],
                            vext[:, 256 * j + 128 * h : 256 * j + 128 * (h + 1)],
                            pt2[:, 512 * h : 512 * (h + 1)],
                            start=(j == 0),
                            stop=(j == nts - 1),
                        )
                return pyh

            def emit_tail(c, pyh):
                # y/Z locations: h0 -> y parts 0-63 (pyh0), Z parts 64-127;
                #                h1 -> Z parts 0-63 (pyh1), y parts 64-127.
                # 1/Z on DVE stays partition-aligned; the cross-partition move
                # of 1/Z rides a SBUF-to-SBUF DMA (no compute engine).
                zcat = smp.tile([128, 512], F32, name=f"zcat_{c}", tag="zcat", bufs=2)
                zal = smp.tile([128, 512], F32, name=f"zal_{c}", tag="zal", bufs=2)
                # pack both heads' Z into one tile (cross-partition copies) so
                # a single FD=512 reciprocal covers them, landing each 1/Z
                # half exactly where the aligned y-multiply needs it
                nc.vector.tensor_copy(zcat[0:64, :], pyh[0][64:128, :])
                nc.vector.tensor_copy(zcat[64:128, :], pyh[1][0:64, :])
                nc.vector.reciprocal(zal[:], zcat[:])
                yt = atp.tile([128, 512], BF16, name=f"yt_{c}", tag="yt", bufs=2)
                nc.vector.tensor_mul(yt[0:64, :], pyh[0][0:64, :], zal[0:64, :])
                nc.vector.tensor_mul(yt[64:128, :], pyh[1][64:128, :], zal[64:128, :])
                for mi in range(4):
                    ms = slice(128 * mi, 128 * (mi + 1))
                    ost = atp.tile([128, D], F32, name=f"ost_{c}_{mi}", tag="ost",
                                   bufs=3)
                    r0 = 512 * c + 128 * mi
                    po = psp.tile([128, D], F32, name=f"po_{c}_{mi}", tag="ps")
                    for nn in range(2):
                        nc.tensor.matmul(
                            po[:, 512 * nn : 512 * (nn + 1)],
                            yt[:, ms],
                            wo_s[:, 512 * nn : 512 * (nn + 1)],
                            start=True,
                            stop=True,
                        )
                    nc.vector.tensor_copy(ost[:], po[:])
                    nc.sync.dma_start(out_d[r0 : r0 + 128, :], ost[:])

            prev = None
            for c in range(NT):
                cur = attention_core(c)
                if prev is not None:
                    emit_tail(c - 1, prev)
                prev = cur
            emit_tail(NT - 1, prev)

    return nc


_NC = None
_NC_SPLIT = False


def _host_shards(x, Wq, Wk, Wv, Wo, s_qk):
    x = np.asarray(x, dtype=np.float32)
    Wq = np.asarray(Wq, dtype=np.float32)
    Wk = np.asarray(Wk, dtype=np.float32)
    Wv = np.asarray(Wv, dtype=np.float32)
    Wo = np.asarray(Wo, dtype=np.float32)
    s_qk = np.asarray(s_qk, dtype=np.float32)

    xT = np.ascontiguousarray(x.reshape(T, D).T).astype(NPBF16)

    dim_q = HD // 4
    freq = (1.0 / 1024.0) ** np.linspace(0.0, 1.0, dim_q, dtype=np.float32)
    freq = np.concatenate([freq, np.zeros(dim_q, np.float32)])
    theta = np.arange(T, dtype=np.float32)[:, None] * freq[None, :]
    cosT = np.cos(theta).T.astype(np.float32)
    sinT = np.sin(theta).T.astype(np.float32)
    A64 = np.concatenate([cosT, cosT], 0)
    B64 = np.concatenate([sinT, -sinT], 0)
    s_eff = s_qk * np.float32(np.sqrt(D))

    hselw = np.zeros((128, 128), np.float32)
    for h in range(HPC):
        hselw[64 * h : 64 * (h + 1), 64 * h : 64 * (h + 1)] = 1.0
    hselw = hselw.astype(NPBF16)

    def wlayout(w):
        # device lhsT chunk i = w_dev[:, 128*i:128*(i+1)] must equal
        # W[128*i + p, f]; store as [p, (i f)] so the DRAM load is contiguous
        return np.ascontiguousarray(
            w.reshape(NK, 128, EPC).transpose(1, 0, 2).reshape(128, NK * EPC)
        ).astype(NPBF16)

    # hswap.T @ raw = swapped raw: dest rows [0:32]<-[32:64], [32:64]<-[0:32],
    # [64:96]<-[96:128], [96:128]<-[64:96]
    hswap = np.zeros((128, 128), np.float32)
    for (a, b) in ((0, 32), (32, 0), (64, 96), (96, 64)):
        for r in range(32):
            hswap[b + r, a + r] = 1.0
    hswap = hswap.astype(NPBF16)

    in_maps = []
    for c in range(NCORES):
        cols = slice(EPC * c, EPC * (c + 1))
        rota_rows, rotb_rows = [], []
        for h in range(HPC):
            s = s_eff[HPC * c + h]
            s_swap = np.concatenate([s[32:], s[:32]])
            rota_rows.append(s[:, None] * A64)
            rotb_rows.append(s_swap[:, None] * B64)
        in_maps.append(
            {
                "xT": xT,
                "wq": wlayout(Wq[:, cols]),
                "wk": wlayout(Wk[:, cols]),
                "wv": wlayout(Wv[:, cols]),
                "wo": np.ascontiguousarray(Wo[EPC * c : EPC * (c + 1), :]).astype(NPBF16),
                "rota": np.concatenate(rota_rows, 0).astype(NPBF16),
                "rotb": np.concatenate(rotb_rows, 0).astype(NPBF16),
                "hselw": hselw,
                "hswap": hswap,
            }
        )
    return in_maps


def _run_device(in_maps):
    global _NC, _NC_SPLIT
    _register_ntff_hook()
    if _NC is None:
        _NC = _build_nc()
    if not _NC_SPLIT:
        _split_ctrl_waits(_NC)
        _NC_SPLIT = True
    res = run_bass_kernel_spmd(_NC, in_maps, list(range(NCORES)))
    return (
        [np.asarray(r["out"]) for r in res.results],
        res.exec_time_ns,
        res.instructions_and_trace[1] if res.instructions_and_trace else None,
    )


def _worker(in_pkl, out_pkl):
    import pickle

    with open(in_pkl, "rb") as f:
        in_maps = pickle.load(f)
    outs, exec_ns, trace = _run_device(in_maps)
    with open(out_pkl, "wb") as f:
        pickle.dump({"outs": outs, "exec_time_ns": exec_ns, "trace": trace}, f)


def _run_subprocess(in_maps):
    import pickle
    import subprocess
    import tempfile

    d = tempfile.mkdtemp()
    in_pkl = os.path.join(d, "in.pkl")
    out_pkl = os.path.join(d, "out.pkl")
    with open(in_pkl, "wb") as f:
        pickle.dump(in_maps, f)
    here = os.path.dirname(os.path.abspath(__file__))
    code = (
        f"import sys; sys.path.insert(0, {here!r}); "
        f"import kernel; kernel._worker({in_pkl!r}, {out_pkl!r})"
    )
    subprocess.run([sys.executable, "-c", code], check=True, timeout=1800)
    with open(out_pkl, "rb") as f:
        out = pickle.load(f)
    return out["outs"], out["exec_time_ns"], out["trace"]


def _attempt(in_maps, use_subprocess):
    if use_subprocess:
        return _run_subprocess(in_maps)
    return _run_device(in_maps)


def kernel(x, Wq, Wk, Wv, Wo, s_qk):
    in_maps = _host_shards(x, Wq, Wk, Wv, Wo, s_qk)

    def total_of(outs):
        t = np.zeros((T, D), np.float64)
        for o in outs:
            t += o.astype(np.float64)
        return t

    # Run until two executions agree: device runs are deterministic, so a
    # mismatch flags the sporadic silent-corruption failure mode. Crashed
    # runs (NRT unrecoverable) poison this process's PJRT client, so later
    # attempts fall back to fresh subprocesses.
    results = []
    last_exc = None
    sub = False
    for attempt in range(5):
        try:
            outs, exec_ns, trace = _attempt(in_maps, sub)
        except Exception as e:
            last_exc = e
            sub = True
            continue
        t = total_of(outs)
        LAST["exec_time_ns"] = exec_ns
        LAST["trace"] = trace
        for tprev in results:
            denom = max(float(np.abs(tprev).max()), 1e-6)
            if float(np.abs(t - tprev).max()) <= 1e-4 * denom:
                return t.astype(np.float32).reshape(1, T, D)
        results.append(t)
    if results:
        return results[-1].astype(np.float32).reshape(1, T, D)
    raise last_exc



# revision 3
# speedup vs baseline: 1.0474x; 1.0474x over previous
"""Trainium2 Bass kernel for nn_CausalSelfAttention (tensor-parallel over heads, 8 cores).

Contract: kernel(**inputs) takes FULL unsharded numpy inputs and returns the
FULL output [1, 2048, 1024] float32. Internally: shards over 8 NeuronCores
(2 heads each, Wq/Wk/Wv column-sharded, Wo row-sharded), runs one SPMD Bass
program via run_bass_kernel_spmd, and sums the 8 partial Wo products on the
host (the row-parallel unshard).

v3 design (v2 was PE-starved: 58% busy, HAM-throttled to 1.2GHz half the
time, 14us serial tail). Changes:
  - warmup dummy matmuls at t0 so the PE HAM clock-gate opens before real
    data lands, and stays open through the DMA-bound projection phase
  - x DMA'd as 4x 1MB chunk-pairs (contiguous, better descriptor economy);
    weights/rot tables ride the second HWDGE ring (nc.scalar) in parallel
  - q and k projections interleaved per chunk-pair (q accumulates in the
    2-bank "ps" slots, k in 4x 1-bank half tiles) so the PE consumes each
    pair at the DMA arrival rate; v projection runs dense from SBUF after
  - rotary pairs are host-permuted to adjacent partitions so the rotary
    partner swap is a single DVE stream_shuffle (no PE hswap matmuls)
  - S/exp for chunk 0 emitted before the v projection for an earlier ACT
    (exp) ramp; exp is the long pole in the attention phase
  - diagonal 512-blocks: S and AV restricted to the causal column range,
    exp via a strided 2-head AP over the valid range, causal mask shrunk to
    one [128,128] affine_select triangle per head
  - Wo partials go to dedicated PSUM half-bank slots (no false deps against
    the S ring); 1/Z via reciprocal_approx_fast (5x); final-chunk tail
    pipelined per 128-row block with PSUM->SBUF copies split DVE/ACT
"""

import os
import sys
import types

import numpy as np
import ml_dtypes

for _p in ("/opt/trn_rl_repo", "/root/.axon_site/_ro/trn_rl_repo"):
    if os.path.isdir(_p) and _p not in sys.path:
        sys.path.append(_p)

import concourse.bass as bass
import concourse.mybir as mybir
import concourse.tile as tile
from concourse.bass_utils import run_bass_kernel_spmd

F32 = mybir.dt.float32
BF16 = mybir.dt.bfloat16
NPBF16 = ml_dtypes.bfloat16
NCORES = 8
T = 2048
D = 1024
NH = 16
HD = 64
HPC = NH // NCORES   # heads per core
EPC = HPC * HD       # projection cols per core
ATTN_SCALE = 0.12
NT = T // 512
NK = D // 128
NDUMMY = 36          # warmup matmuls (N=128) to open the HAM clock gate

# stream_shuffle swaps adjacent partitions within each 32-partition quadrant;
# the host layout puts each rotary pair (d, d+32) on adjacent partitions.
SWAP_MASK = [(i ^ 1) for i in range(32)]

LAST = {}


def _register_ntff_hook():
    """Best-effort: register the axon NTFF profile hook if the image's antenv
    lacks axon_hooks (profiling only; compile/run work without it)."""
    try:
        import antenv.axon_hooks  # noqa: F401
        return
    except ImportError:
        pass
    try:
        import trn_agent_boot.trn_boot as tb

        mod = types.ModuleType("antenv.axon_hooks")
        holder = {}
        mod.set_axon_ntff_profile_hook = lambda h: holder.__setitem__("h", h)
        mod.get_axon_ntff_profile_hook = lambda: holder.get("h")
        sys.modules["antenv.axon_hooks"] = mod
        mod.set_axon_ntff_profile_hook(
            tb._ntff_profile_via_ctypes("/opt/axon/libaxon_pjrt.so")
        )
    except Exception:
        pass


def _split_ctrl_waits(nc, k_default=1):
    """The container's walrus build rejects instructions carrying more than one
    semaphore sync-wait; hoist extra waits onto single-wait NoOps that precede
    the instruction on the same engine queue (AND semantics preserved)."""
    n_nops = 0
    for f in nc.m.functions:
        for blk in f.blocks:
            new, changed = [], False
            for inst in list(blk.instructions):
                si = inst.sync_info
                waits = list(si.on_wait) if si is not None else []
                kmax = 1 if isinstance(inst, mybir.InstDrain) else k_default
                if len(waits) > kmax:
                    for k, w in enumerate(waits[:-kmax]):
                        nop = mybir.InstNoOp(name=f"{inst.name}-sw{k}", ins=[], outs=[])
                        nop.engine = inst.engine
                        nop.sync_info = mybir.SyncInfo(on_wait=[w], on_update=[])
                        new.append(nop)
                        n_nops += 1
                    inst.sync_info = mybir.SyncInfo(
                        on_wait=list(waits[-kmax:]), on_update=list(si.on_update)
                    )
                    changed = True
                new.append(inst)
            if changed:
                blk.instructions = new
    return n_nops


def _build_nc():
    nc = bass.Bass("TRN2", target_bir_lowering=False, debug=False, num_devices=NCORES)

    # x packed host-side as [128, (pair, i, t)]: pair-major so each 1MB pair
    # load is one fully contiguous 8KB-per-partition DMA.
    xP_d = nc.dram_tensor("xP", [128, 4 * 2 * T], BF16, kind="ExternalInput")
    wq_d = nc.dram_tensor("wq", [128, D], BF16, kind="ExternalInput")
    wk_d = nc.dram_tensor("wk", [128, D], BF16, kind="ExternalInput")
    wv_d = nc.dram_tensor("wv", [128, D], BF16, kind="ExternalInput")
    wo_d = nc.dram_tensor("wo", [EPC, D], BF16, kind="ExternalInput")
    rota_d = nc.dram_tensor("rota", [EPC, T], BF16, kind="ExternalInput")
    rotb_d = nc.dram_tensor("rotb", [EPC, T], BF16, kind="ExternalInput")
    hselw_d = nc.dram_tensor("hselw", [128, 128], BF16, kind="ExternalInput")
    out_d = nc.dram_tensor("out", [T, D], F32, kind="ExternalOutput")

    with tile.TileContext(nc) as tc:
        with (
            tc.tile_pool(name="wt", bufs=1) as wt,
            tc.tile_pool(name="big", bufs=4) as big,      # x chunk-pairs
            tc.tile_pool(name="praw", bufs=3) as prawp,   # q/k/v raw
            tc.tile_pool(name="lwp", bufs=2) as lwp,      # ln(sumsq) staging
            tc.tile_pool(name="rwp", bufs=2) as rwp,      # rsqrt scales
            tc.tile_pool(name="rotp", bufs=2) as rotp,    # qrot/krot
            tc.tile_pool(name="sm", bufs=2) as smp,       # misc small tiles
            tc.tile_pool(name="at", bufs=1) as atp,       # pt2 / yt / ost
            # PSUM: "ps" 2x[128,1024] (4 banks) + "h0","h1" 2x[128,512] each
            # (2+2 banks) = 8 banks exactly.
            tc.tile_pool(name="ps", bufs=2, space="PSUM") as psp,
            tc.tile_pool(name="h0", bufs=2, space="PSUM") as h0p,
            tc.tile_pool(name="h1", bufs=2, space="PSUM") as h1p,
        ):
            # ---- ACT table warmup + PE warmup ----
            eps = wt.tile([128, 1], F32, tag="eps")
            nc.gpsimd.memset(eps[:], 1e-12)
            warm_in = wt.tile([128, 128], BF16, tag="warm_in")
            nc.gpsimd.memset(warm_in[:], 0.001)
            warm_f = wt.tile([128, 2], F32, tag="warm_f")
            warm_g = wt.tile([128, 2], F32, tag="warm_g")
            nc.gpsimd.memset(warm_f[:], 1.0)
            nc.scalar.activation(
                warm_g[:], warm_f[:], mybir.ActivationFunctionType.Ln,
                bias=eps[:],
            )
            nc.scalar.activation(
                warm_f[:], warm_g[:], mybir.ActivationFunctionType.Exp
            )
            # dummy matmuls: no data deps; keep the PE busy (and the HAM
            # un-throttled) from the preamble until the first x pair lands
            warm_ps = psp.tile([128, 1024], F32, tag="ps", name="warm_ps")
            for d in range(NDUMMY):
                nc.tensor.matmul(
                    warm_ps[:, 0:128], warm_in[:], warm_in[:],
                    start=True, stop=True,
                )

            # ---- constants / weights on the second HWDGE ring (scalar) ----
            wq_s = wt.tile([128, D], BF16, tag="wq")
            wk_s = wt.tile([128, D], BF16, tag="wk")
            wv_s = wt.tile([128, D], BF16, tag="wv")
            wo_s = wt.tile([EPC, D], BF16, tag="wo")
            rota = wt.tile([EPC, T], BF16, tag="rota")
            rotb = wt.tile([EPC, T], BF16, tag="rotb")
            hselw = wt.tile([128, 128], BF16, tag="hselw")
            vext = wt.tile([128, (T // 128) * 256], BF16, tag="vext")
            nc.scalar.dma_start(wq_s[:], wq_d[:])
            nc.scalar.dma_start(wk_s[:], wk_d[:])
            nc.scalar.dma_start(hselw[:], hselw_d[:])
            nc.scalar.dma_start(wv_s[:], wv_d[:])
            nc.scalar.dma_start(rota[:], rota_d[:])
            nc.scalar.dma_start(rotb[:], rotb_d[:])
            nc.scalar.dma_start(wo_s[:], wo_d[:])
            nc.gpsimd.memset(vext[:], 1.0)

            ident = wt.tile([128, 128], BF16, tag="ident")
            nc.gpsimd.memset(ident[:], 0.0)
            nc.gpsimd.affine_select(
                out=ident[:],
                in_=ident[:],
                compare_op=mybir.AluOpType.not_equal,
                fill=1.0,
                base=0,
                pattern=[[-1, 128]],
                channel_multiplier=1,
            )

            # ---- interleaved q+k projections, chunk-pair streamed ----
            xcp = []
            for a in range(4):
                t_ = big.tile([128, 2 * T], BF16, tag="big", name=f"xp_{a}")
                nc.sync.dma_start(t_[:], xP_d[:, 2 * T * a : 2 * T * (a + 1)])
                xcp.append(t_)

            pq = [
                psp.tile([128, 1024], F32, tag="ps", name=f"pq_{p}")
                for p in range(2)
            ]
            pk = [
                h0p.tile([128, 512], F32, tag="h0", name="pk_0"),
                h0p.tile([128, 512], F32, tag="h0", name="pk_1"),
                h1p.tile([128, 512], F32, tag="h1", name="pk_2"),
                h1p.tile([128, 512], F32, tag="h1", name="pk_3"),
            ]
            for a in range(4):
                for b in range(2):
                    i = 2 * a + b
                    xi = xcp[a][:, 2048 * b : 2048 * (b + 1)]
                    wsl = slice(128 * i, 128 * (i + 1))
                    for n in range(NT):
                        nc.tensor.matmul(
                            pq[n // 2][:, 512 * (n % 2) : 512 * (n % 2 + 1)],
                            wq_s[:, wsl],
                            xi[:, 512 * n : 512 * (n + 1)],
                            start=(i == 0),
                            stop=(i == NK - 1),
                        )
                    for n in range(NT):
                        nc.tensor.matmul(
                            pk[n][:],
                            wk_s[:, wsl],
                            xi[:, 512 * n : 512 * (n + 1)],
                            start=(i == 0),
                            stop=(i == NK - 1),
                        )

            q_raw = prawp.tile([EPC, T], BF16, tag="praw", name="raw_q")
            k_raw = prawp.tile([EPC, T], BF16, tag="praw", name="raw_k")
            for p in range(2):
                nc.vector.tensor_copy(
                    q_raw[:, 1024 * p : 1024 * (p + 1)], pq[p][:]
                )
            for n in range(NT):
                nc.vector.tensor_copy(
                    k_raw[:, 512 * n : 512 * (n + 1)], pk[n][:]
                )

            # ---- cosine-norm + scale + rotary ----
            # rsqrt(sumsq) = exp(-0.5 ln(sumsq)); Ln and Exp share the
            # attention Exp table set (no table switch). Rotary partner swap
            # is a DVE stream_shuffle (host permuted pairs to adjacency).
            def norm_rot(raw, tname):
                lw = lwp.tile([128, T], F32, tag="lw", name=f"lw_{tname}")
                t2s = []
                for n in range(NT):
                    sl = slice(512 * n, 512 * (n + 1))
                    sw = smp.tile([128, 512], BF16, name=f"sw_{tname}_{n}",
                                  tag="sw", bufs=4)
                    nc.vector.stream_shuffle(sw[:], raw[:, sl], SWAP_MASK)
                    sq = smp.tile([128, 512], BF16, name=f"sq_{tname}_{n}",
                                  tag="sqm", bufs=4)
                    nc.gpsimd.tensor_mul(sq[:], raw[:, sl], raw[:, sl])
                    ssb = psp.tile([128, 512], F32, name=f"ssb_{tname}_{n}",
                                   tag="ps")
                    nc.tensor.matmul(ssb[:], hselw[:], sq[:], start=True,
                                     stop=True)
                    nc.scalar.activation(
                        lw[:, sl], ssb[:], mybir.ActivationFunctionType.Ln,
                        bias=eps[:],
                    )
                    t2 = smp.tile([128, 512], BF16, name=f"t2_{tname}_{n}",
                                  tag="t2", bufs=4)
                    nc.gpsimd.tensor_mul(t2[:], sw[:], rotb[:, sl])
                    t2s.append(t2)
                rw = rwp.tile([128, T], BF16, tag="rw", name=f"rw_{tname}")
                nc.scalar.activation(
                    rw[:, 0:512], lw[:, 0:512],
                    mybir.ActivationFunctionType.Exp, scale=-0.5,
                )
                nc.scalar.activation(
                    rw[:, 512:T], lw[:, 512:T],
                    mybir.ActivationFunctionType.Exp, scale=-0.5,
                )
                rot = rotp.tile([EPC, T], BF16, tag="rot", name=f"rot_{tname}")
                for n in range(NT):
                    sl = slice(512 * n, 512 * (n + 1))
                    t1 = smp.tile([128, 512], BF16, name=f"t1_{tname}_{n}",
                                  tag="t1", bufs=4)
                    nc.vector.tensor_mul(t1[:], raw[:, sl], rota[:, sl])
                    nc.vector.tensor_add(t1[:], t1[:], t2s[n][:])
                    nc.vector.tensor_mul(rot[:, sl], t1[:], rw[:, sl])
                return rot

            qrot = norm_rot(q_raw, "q")
            krot = norm_rot(k_raw, "k")

            # ---- attention pieces (S^T layout) ----
            pt_tiles = {}

            def emit_s(c, j):
                # ps2[:, (h, q)]: partition = k-time within block j. Diagonal
                # blocks only compute the causal column range [128m, 512).
                m = j - 4 * c
                lo = 128 * m if m > 0 else 0
                cq = slice(512 * c + lo, 512 * (c + 1))
                ps2 = psp.tile([128, 1024], F32, tag="ps", name=f"s_{c}_{j}")
                for h in range(HPC):
                    hs = slice(64 * h, 64 * (h + 1))
                    nc.tensor.matmul(
                        ps2[:, 512 * h + lo : 512 * (h + 1)],
                        krot[hs, 128 * j : 128 * (j + 1)],
                        qrot[hs, cq],
                        start=True,
                        stop=True,
                        tile_position=(64 * h, 0),
                    )
                return ps2

            def emit_exp(c, j, ps2):
                m = j - 4 * c
                pt2 = atp.tile([128, 1024], BF16, tag="pt", bufs=8,
                               name=f"p_{c}_{j}")
                if m > 0:
                    src = ps2.rearrange("p (h q) -> p h q", h=2)[:, :, 128 * m : 512]
                    dst = pt2.rearrange("p (h q) -> p h q", h=2)[:, :, 128 * m : 512]
                else:
                    src, dst = ps2[:], pt2[:]
                nc.scalar.activation(
                    dst, src, mybir.ActivationFunctionType.Exp,
                    scale=ATTN_SCALE,
                )
                if m >= 0:
                    # causal mask only on the [128,128] diagonal triangle
                    for h in range(HPC):
                        dsl = slice(512 * h + 128 * m, 512 * h + 128 * m + 128)
                        nc.gpsimd.affine_select(
                            out=pt2[:, dsl],
                            in_=pt2[:, dsl],
                            compare_op=mybir.AluOpType.is_ge,
                            fill=0.0,
                            base=0,
                            pattern=[[1, 128]],
                            channel_multiplier=-1,
                        )
                pt_tiles[(c, j)] = pt2

            def emit_av(c, j, pyh, nts):
                m = j - 4 * c
                lo = 128 * m if m > 0 else 0
                pt2 = pt_tiles.pop((c, j))
                for h in range(HPC):
                    nc.tensor.matmul(
                        pyh[h][:, lo:512],
                        vext[:, 256 * j + 128 * h : 256 * j + 128 * (h + 1)],
                        pt2[:, 512 * h + lo : 512 * (h + 1)],
                        start=(j == 0),
                        stop=(j == nts - 1),
                        skip_group_check=True,
                    )

            # ---- chunk 0 S/exp first: feeds ACT (the attention-phase
            # bottleneck) while the PE still runs the v projection ----
            s_pend = {}
            for j in range(4):
                s_pend[(0, j)] = emit_s(0, j)
                emit_exp(0, j, s_pend.pop((0, j)))

            # ---- v projection (dense, x already in SBUF) ----
            pv = [
                h0p.tile([128, 512], F32, tag="h0", name="pv_0"),
                h0p.tile([128, 512], F32, tag="h0", name="pv_1"),
                h1p.tile([128, 512], F32, tag="h1", name="pv_2"),
                h1p.tile([128, 512], F32, tag="h1", name="pv_3"),
            ]
            for a in range(4):
                for b in range(2):
                    i = 2 * a + b
                    xi = xcp[a][:, 2048 * b : 2048 * (b + 1)]
                    wsl = slice(128 * i, 128 * (i + 1))
                    for n in range(NT):
                        nc.tensor.matmul(
                            pv[n][:],
                            wv_s[:, wsl],
                            xi[:, 512 * n : 512 * (n + 1)],
                            start=(i == 0),
                            stop=(i == NK - 1),
                        )
            vT_raw = prawp.tile([EPC, T], BF16, tag="praw", name="raw_v")
            for n in range(NT):
                nc.vector.tensor_copy(
                    vT_raw[:, 512 * n : 512 * (n + 1)], pv[n][:]
                )

            # ---- v to natural layout via PE transpose ----
            # vext per head h0=[v|ones], h1=[ones|v]: the AV matmul then puts
            # y and Z on partition halves that keep later DVE ops aligned.
            for j in range(T // 128):
                pool = h0p if (j % 2 == 0) else h1p
                tp_ = pool.tile([128, 128], BF16, tag=("h0" if j % 2 == 0 else "h1"),
                                name=f"vtp_{j}")
                nc.tensor.transpose(
                    tp_[:], vT_raw[:, 128 * j : 128 * (j + 1)], ident[:]
                )
                nc.vector.tensor_copy(
                    vext[:, 256 * j : 256 * j + 64], tp_[:, 0:64]
                )
                nc.vector.tensor_copy(
                    vext[:, 256 * j + 192 : 256 * j + 256], tp_[:, 64:128]
                )

            # ---- attention core / tails ----
            def attention_core(c, skip_lead=0):
                # Software-pipelined: S/exp for block j+PF emitted before
                # mask/AV of block j so PE/ACT/GpSimd all stream.
                PF = 4
                nts = 4 * c + 4
                pyh = [
                    h1p.tile([128, 512], F32, name=f"py0_{c}", tag="h1"),
                    h1p.tile([128, 512], F32, name=f"py1_{c}", tag="h1"),
                ]
                for j in range(skip_lead, min(PF, nts)):
                    s_pend[(c, j)] = emit_s(c, j)
                    emit_exp(c, j, s_pend.pop((c, j)))
                for j in range(nts):
                    if j + PF < nts:
                        s_pend[(c, j + PF)] = emit_s(c, j + PF)
                        emit_exp(c, j + PF, s_pend.pop((c, j + PF)))
                    emit_av(c, j, pyh, nts)
                return pyh

            def emit_tail(c, pyh, last=False):
                # y/Z locations: h0 -> y parts 0-63 (pyh0), Z parts 64-127;
                #                h1 -> Z parts 0-63 (pyh1), y parts 64-127.
                zcat = smp.tile([128, 512], F32, name=f"zcat_{c}", tag="zcat",
                                bufs=2)
                zal = smp.tile([128, 512], F32, name=f"zal_{c}", tag="zal",
                               bufs=2)
                nc.vector.tensor_copy(zcat[0:64, :], pyh[0][64:128, :])
                nc.vector.tensor_copy(zcat[64:128, :], pyh[1][0:64, :])
                if last:
                    # ACT is idle after the final exp; 1/Z = exp(-ln(Z)) via
                    # the already-loaded Ln/Exp tables is ~4x faster than the
                    # DVE reciprocal and shortens the end-of-kernel chain
                    zlog = smp.tile([128, 512], F32, name=f"zlog_{c}",
                                    tag="zlog", bufs=1)
                    nc.scalar.activation(
                        zlog[:], zcat[:], mybir.ActivationFunctionType.Ln,
                        bias=eps[:],
                    )
                    nc.scalar.activation(
                        zal[:], zlog[:], mybir.ActivationFunctionType.Exp,
                        scale=-1.0,
                    )
                else:
                    nc.vector.reciprocal(zal[:], zcat[:])
                yt = atp.tile([128, 512], BF16, name=f"yt_{c}", tag="yt", bufs=2)
                nc.vector.tensor_mul(yt[0:64, :], pyh[0][0:64, :], zal[0:64, :])
                nc.vector.tensor_mul(yt[64:128, :], pyh[1][64:128, :],
                                     zal[64:128, :])
                for mi in range(4):
                    ms = slice(128 * mi, 128 * (mi + 1))
                    ost = atp.tile([128, D], F32, name=f"ost_{c}_{mi}", tag="ost",
                                   bufs=3)
                    r0 = 512 * c + 128 * mi
                    for nn in range(2):
                        pool = h0p if (mi % 2 == 0) else h1p
                        po = pool.tile([128, 512], F32, name=f"po_{c}_{mi}_{nn}",
                                       tag=("h0" if mi % 2 == 0 else "h1"))
                        nc.tensor.matmul(
                            po[:],
                            yt[:, ms],
                            wo_s[:, 512 * nn : 512 * (nn + 1)],
                            start=True,
                            stop=True,
                        )
                        osl = slice(512 * nn, 512 * (nn + 1))
                        if last and (mi + nn) % 2 == 1:
                            nc.scalar.copy(ost[:, osl], po[:])
                        else:
                            nc.vector.tensor_copy(ost[:, osl], po[:])
                    nc.sync.dma_start(out_d[r0 : r0 + 128, :], ost[:])

            prev = attention_core(0, skip_lead=4)
            for c in range(1, NT):
                cur = attention_core(c)
                emit_tail(c - 1, prev)
                prev = cur
            emit_tail(NT - 1, prev, last=True)

    return nc


_NC = None
_NC_SPLIT = False


def _host_shards(x, Wq, Wk, Wv, Wo, s_qk):
    x = np.asarray(x, dtype=np.float32)
    Wq = np.asarray(Wq, dtype=np.float32)
    Wk = np.asarray(Wk, dtype=np.float32)
    Wv = np.asarray(Wv, dtype=np.float32)
    Wo = np.asarray(Wo, dtype=np.float32)
    s_qk = np.asarray(s_qk, dtype=np.float32)

    xT = np.ascontiguousarray(x.reshape(T, D).T).astype(NPBF16)
    # pack as [128, (pair, i_in_pair, t)] so each pair is one contiguous DMA
    xP = np.ascontiguousarray(
        xT.reshape(4, 2, 128, T).transpose(2, 0, 1, 3).reshape(128, 8 * T)
    )

    dim_q = HD // 4
    freq = (1.0 / 1024.0) ** np.linspace(0.0, 1.0, dim_q, dtype=np.float32)
    freq = np.concatenate([freq, np.zeros(dim_q, np.float32)])
    theta = np.arange(T, dtype=np.float32)[:, None] * freq[None, :]
    cosT = np.cos(theta).T.astype(np.float32)
    sinT = np.sin(theta).T.astype(np.float32)
    A64 = np.concatenate([cosT, cosT], 0)          # [64, T]
    B64 = np.concatenate([sinT, -sinT], 0)         # [64, T]
    s_eff = s_qk * np.float32(np.sqrt(D))

    # per-head partition permutation: device partition j holds source dim
    # d(j) = (j%2)*32 + j//2, so rotary pairs (d, d+32) sit on (2r, 2r+1)
    dperm = np.array([(j % 2) * 32 + j // 2 for j in range(HD)], np.int64)
    dperm_sw = dperm[np.arange(HD) ^ 1]

    hselw = np.zeros((128, 128), np.float32)
    for h in range(HPC):
        hselw[64 * h : 64 * (h + 1), 64 * h : 64 * (h + 1)] = 1.0
    hselw = hselw.astype(NPBF16)

    def wlayout(w):
        # device lhsT chunk i = w_dev[:, 128*i:128*(i+1)] must equal
        # W[128*i + p, f]; store as [p, (i f)] so the DRAM load is contiguous
        return np.ascontiguousarray(
            w.reshape(NK, 128, EPC).transpose(1, 0, 2).reshape(128, NK * EPC)
        ).astype(NPBF16)

    in_maps = []
    for c in range(NCORES):
        cols = slice(EPC * c, EPC * (c + 1))
        wq_c = Wq[:, cols].reshape(D, HPC, HD)[:, :, dperm].reshape(D, EPC)
        wk_c = Wk[:, cols].reshape(D, HPC, HD)[:, :, dperm].reshape(D, EPC)
        rota_rows, rotb_rows = [], []
        for h in range(HPC):
            s = s_eff[HPC * c + h]
            rota_rows.append(s[dperm][:, None] * A64[dperm])
            rotb_rows.append(s[dperm_sw][:, None] * B64[dperm])
        in_maps.append(
            {
                "xP": xP,
                "wq": wlayout(wq_c),
                "wk": wlayout(wk_c),
                "wv": wlayout(Wv[:, cols]),
                "wo": np.ascontiguousarray(Wo[EPC * c : EPC * (c + 1), :]).astype(NPBF16),
                "rota": np.concatenate(rota_rows, 0).astype(NPBF16),
                "rotb": np.concatenate(rotb_rows, 0).astype(NPBF16),
                "hselw": hselw,
            }
        )
    return in_maps


def _run_device(in_maps):
    global _NC, _NC_SPLIT
    _register_ntff_hook()
    if _NC is None:
        _NC = _build_nc()
    if not _NC_SPLIT:
        _split_ctrl_waits(_NC)
        _NC_SPLIT = True
    res = run_bass_kernel_spmd(_NC, in_maps, list(range(NCORES)))
    return (
        [np.asarray(r["out"]) for r in res.results],
        res.exec_time_ns,
        res.instructions_and_trace[1] if res.instructions_and_trace else None,
    )


def _worker(in_pkl, out_pkl):
    import pickle

    with open(in_pkl, "rb") as f:
        in_maps = pickle.load(f)
    outs, exec_ns, trace = _run_device(in_maps)
    with open(out_pkl, "wb") as f:
        pickle.dump({"outs": outs, "exec_time_ns": exec_ns, "trace": trace}, f)


def _run_subprocess(in_maps):
    import pickle
    import subprocess
    import tempfile

    d = tempfile.mkdtemp()
    in_pkl = os.path.join(d, "in.pkl")
    out_pkl = os.path.join(d, "out.pkl")
    with open(in_pkl, "wb") as f:
        pickle.dump(in_maps, f)
    here = os.path.dirname(os.path.abspath(__file__))
    code = (
        f"import sys; sys.path.insert(0, {here!r}); "
        f"import kernel; kernel._worker({in_pkl!r}, {out_pkl!r})"
    )
    subprocess.run([sys.executable, "-c", code], check=True, timeout=1800)
    with open(out_pkl, "rb") as f:
        out = pickle.load(f)
    return out["outs"], out["exec_time_ns"], out["trace"]


def _attempt(in_maps, use_subprocess):
    if use_subprocess:
        return _run_subprocess(in_maps)
    return _run_device(in_maps)


def kernel(x, Wq, Wk, Wv, Wo, s_qk):
    in_maps = _host_shards(x, Wq, Wk, Wv, Wo, s_qk)

    def total_of(outs):
        t = np.zeros((T, D), np.float64)
        for o in outs:
            t += o.astype(np.float64)
        return t

    # Run until two executions agree: device runs are deterministic, so a
    # mismatch flags the sporadic silent-corruption failure mode. Crashed
    # runs (NRT unrecoverable) poison this process's PJRT client, so later
    # attempts fall back to fresh subprocesses.
    results = []
    last_exc = None
    sub = False
    for attempt in range(5):
        try:
            outs, exec_ns, trace = _attempt(in_maps, sub)
        except Exception as e:
            last_exc = e
            sub = True
            continue
        t = total_of(outs)
        LAST["exec_time_ns"] = exec_ns
        LAST["trace"] = trace
        for tprev in results:
            denom = max(float(np.abs(tprev).max()), 1e-6)
            if float(np.abs(t - tprev).max()) <= 1e-4 * denom:
                return t.astype(np.float32).reshape(1, T, D)
        results.append(t)
    if results:
        return results[-1].astype(np.float32).reshape(1, T, D)
    raise last_exc
